# revision 20
# baseline (speedup 1.0000x reference)
"""Trainium2 Bass kernel for quantized causal self-attention.

Sharding: Megatron-style tensor parallelism over heads. 16 heads are split
across 8 NeuronCores (2 heads/core). Each core computes, for all 4 batches:
  - its QKV head-slice projection (x @ w_slice^T, int8 weights held exactly
    in bf16, quant scales folded into epilogues / host),
  - causal attention for its 2 heads (transposed-scores layout, exp without
    max-subtraction - scores are bounded ~5 for this model family),
  - a partial output projection against its column slice of w_proj.
The host sums the 8 partial projections, applies the commuting quant scales
(s_w_attn * s_w_proj), and adds the biases that commute out of the linear
ops (c_proj bias, and the v-bias term which passes through softmax-normalized
attention as a constant row).

Schedule: fully software-pipelined across batches. Every stage s of batch b
interleaves, in PE program order, work with no mutual dependencies:
  scores(b,h0,row s) | QK(b+1, group s) | scores(b,h1,row s) |
  att@v(b,h0,qb=s) | V(b+1, group s) | att@v(b,h1,qb=s) |
  y-transposes | c_proj(b, tb=s-1)
so the tensor engine never waits on the scalar-engine exp or the DVE drain
chains. The causal-mask multiply runs on the otherwise-idle Pool engine and
the c_proj drains alternate DVE/ACT. Partials are written in bf16.
"""

import numpy as np
import ml_dtypes

B, T, C, H, D = 4, 1024, 2048, 16, 128
NCORES = 8
HPC = H // NCORES          # heads per core = 2
CS = HPC * D               # per-core head feature slice = 256
BT = B * T                 # 4096 tokens
TB = T // 128              # 8 token blocks per batch
CCH = C // 128             # 16 contraction chunks

BF16 = ml_dtypes.bfloat16

_CACHE = {}


def _build_program(s_wa: float):
    import concourse.tile as tile
    from concourse import bacc, mybir
    from concourse.masks import make_identity, make_upper_triangular

    f32 = mybir.dt.float32
    bf16 = mybir.dt.bfloat16
    AF = mybir.ActivationFunctionType
    inv_sqrt_d = 1.0 / float(np.sqrt(D))

    nc = bacc.Bacc("TRN2", target_bir_lowering=False, debug=False)

    # all partition-major so SBUF-shaped multi-chunk DMAs need no transpose
    xT = nc.dram_tensor("xT", [B, 128, CCH, T], bf16, kind="ExternalInput")
    wqk = nc.dram_tensor("wqk", [128, CCH, 4 * 128], bf16, kind="ExternalInput")
    wv = nc.dram_tensor("wv", [128, CCH, CS], bf16, kind="ExternalInput")
    wp = nc.dram_tensor("wp", [HPC, 128, C], bf16, kind="ExternalInput")
    bqk = nc.dram_tensor("bqk", [128, 4], f32, kind="ExternalInput")
    partial = nc.dram_tensor("partial", [BT, C], bf16, kind="ExternalOutput")

    with tile.TileContext(nc) as tc:
        with (
            tc.tile_pool(name="singles", bufs=1) as singles,
            tc.tile_pool(name="xpool", bufs=2) as xpool,
            tc.tile_pool(name="qkpool", bufs=3) as qkpool,
            tc.tile_pool(name="vpool", bufs=3) as vpool,
            tc.tile_pool(name="attpool", bufs=3) as attpool,
            tc.tile_pool(name="ytpool", bufs=4) as ytpool,
            tc.tile_pool(name="ypool", bufs=3) as ypool,
            tc.tile_pool(name="rlpool", bufs=4) as rlpool,
            tc.tile_pool(name="outpool", bufs=4) as outpool,
            # 3 banks shared by QK / V / c_proj matmul groups: their drains
            # complete quickly, so a 3-deep ring never stalls the PE
            tc.tile_pool(name="psbig", bufs=3, space="PSUM") as psbig,
            # score rows: up to 4 chunks in flight per stage while the
            # scalar engine works through the exps
            tc.tile_pool(name="psatt", bufs=3, space="PSUM") as psatt,
            # att@v accumulators (129 cols + bf16 transpose target)
            tc.tile_pool(name="pssm", bufs=2, space="PSUM") as pssm,
        ):
            # weights / constants, resident for the whole kernel
            wqk_s = singles.tile([128, CCH, 4 * 128], bf16, tag="wqk")
            wv_s = singles.tile([128, CCH, CS], bf16, tag="wv")
            wp_s = [singles.tile([128, C], bf16, tag=f"wp{h}", name=f"wp_s{h}")
                    for h in range(HPC)]
            bqk_s = singles.tile([128, 4], f32, tag="bqk")
            trimask = singles.tile([128, 128], bf16, tag="trimask")
            ident = singles.tile([128, 128], bf16, tag="ident")

            # ---- emitters --------------------------------------------------
            def emit_qk_group(xb, qkT, idx, order=2):
                # idx -> (ob, th); ob: 0 = q head0, 1 = q head1, 2 = k
                # head0, 3 = k head1; th: halves of the 1024 tokens.
                # order=1 walks th-major (prologue: matches DMA arrival)
                if order == 2:
                    ob, th = idx // 2, idx % 2
                else:
                    ob, th = idx % 4, idx // 4
                ps = psbig.tile([128, 512], f32, tag="ps", name="ps")
                for g in range(CCH):
                    nc.tensor.matmul(
                        ps[:],
                        wqk_s[:, g, ob * 128:(ob + 1) * 128],
                        xb[:, g, th * 512:(th + 1) * 512],
                        start=(g == 0),
                        stop=(g == CCH - 1),
                    )
                nc.vector.tensor_scalar(
                    qkT[:, ob, th * 512:(th + 1) * 512],
                    ps[:],
                    s_wa,
                    bqk_s[:, ob:ob + 1],
                    mybir.AluOpType.mult,
                    mybir.AluOpType.add,
                )

            def emit_v_group(xb, v_all, tb):
                ps = psbig.tile([128, CS], f32, tag="ps", name="ps")
                for g in range(CCH):
                    nc.tensor.matmul(
                        ps[:],
                        xb[:, g, tb * 128:(tb + 1) * 128],
                        wv_s[:, g, :],
                        start=(g == 0),
                        stop=(g == CCH - 1),
                    )
                nc.vector.tensor_copy(
                    v_all[:, tb, :, 0:D],
                    ps[:].rearrange("p (h d) -> p h d", h=HPC),
                )

            def emit_score_row(qkT, h, attT, kb):
                width = T - kb * 128
                off = 0
                while off < width:
                    w = min(512, width - off)
                    ps = psatt.tile([128, 512], f32, tag="ps", name="ps")
                    nc.tensor.matmul(
                        ps[:, 0:w],
                        qkT[:, 2 + h, kb * 128:(kb + 1) * 128],
                        qkT[:, h, kb * 128 + off:kb * 128 + off + w],
                    )
                    nc.scalar.activation(
                        attT[:, kb, off:off + w],
                        ps[:, 0:w],
                        AF.Exp,
                        scale=inv_sqrt_d,
                    )
                    off += w
                # causal mask on the diagonal block (multiplicative), on the
                # otherwise-idle Pool engine
                nc.gpsimd.tensor_mul(
                    attT[:, kb, 0:128], attT[:, kb, 0:128], trimask[:],
                )

            def emit_av(attT, v_all, h, qb):
                # cols 0:D = y accum, col D = row-sum (ones column of v);
                # the bf16 view of f32 cols 192:256 later receives the PE
                # transpose so no extra PSUM slot is consumed for it
                psy = pssm.tile([128, 256], f32, tag="psy", name="psy")
                for kb in range(qb + 1):
                    nc.tensor.matmul(
                        psy[:, 0:D + 1],
                        attT[:, kb, (qb - kb) * 128:(qb - kb) * 128 + 128],
                        v_all[:, kb, h, :],
                        start=(kb == 0),
                        stop=(kb == qb),
                    )
                rl = rlpool.tile([128, 1], f32, tag="rl", name="rl")
                nc.vector.reciprocal(rl[:], psy[:, D:D + 1])
                ysb = ypool.tile([128, 128], bf16, tag="ysb", name="ysb")
                nc.vector.tensor_scalar_mul(ysb[:], psy[:, 0:D], rl[:])
                return psy, ysb

            def emit_transpose(psy, ysb, yT, qb):
                pst = psy[:, 192:256].bitcast(bf16)
                nc.tensor.transpose(pst, ysb[:], ident[:])
                nc.vector.tensor_copy(yT[:, qb * 128:(qb + 1) * 128], pst)

            def emit_proj(t0, yTs, tb, split_dma=False):
                # ob 0/1 drain on DVE, ob 2/3 on ACT, so the engine queues
                # stay balanced. split_dma halves the store transfers (for
                # the kernel tail, where the last transfer's latency is on
                # the critical path).
                for ob in range(4):
                    ps = psbig.tile([128, 512], f32, tag="ps", name="ps")
                    for h in range(HPC):
                        nc.tensor.matmul(
                            ps[:],
                            yTs[h][:, tb * 128:(tb + 1) * 128],
                            wp_s[h][:, ob * 512:(ob + 1) * 512],
                            start=(h == 0),
                            stop=(h == HPC - 1),
                        )
                    po = outpool.tile([128, 512], bf16, tag="po", name="po")
                    if ob < 2:
                        nc.vector.tensor_copy(po[:], ps[:])
                    else:
                        nc.scalar.copy(po[:], ps[:])
                    rows = partial[t0 + tb * 128:t0 + (tb + 1) * 128]
                    if split_dma:
                        for q in range(2):
                            c0 = ob * 512 + q * 256
                            nc.sync.dma_start(rows[:, c0:c0 + 256],
                                              po[:, q * 256:(q + 1) * 256])
                    else:
                        nc.sync.dma_start(rows[:, ob * 512:(ob + 1) * 512],
                                          po[:])

            # ---- prologue --------------------------------------------------
            # weights dispatch from the (idle) ACT hardware-DGE queue, x from
            # SP. Slice the loads so the first QK group (ob=0, th=0) only
            # needs its own wqk column block + the th=0 half of x: ~2.6MB of
            # HBM traffic instead of 6.2MB before the PE gets rolling.
            xbs = [None] * B
            xbs[0] = xpool.tile([128, CCH, T], bf16, tag="xb", name="xb")
            xbs[1] = xpool.tile([128, CCH, T], bf16, tag="xb", name="xb")
            for ob in range(4):
                nc.scalar.dma_start(wqk_s[:, :, ob * 128:(ob + 1) * 128],
                                    wqk[:, :, ob * 128:(ob + 1) * 128])
            nc.scalar.dma_start(bqk_s[:], bqk[:])
            for th in range(2):
                for a in range(4):
                    nc.sync.dma_start(
                        xbs[0][:, 4 * a:4 * a + 4, th * 512:(th + 1) * 512],
                        xT[0, :, 4 * a:4 * a + 4, th * 512:(th + 1) * 512])
            for a in range(2):
                nc.scalar.dma_start(wv_s[:, 8 * a:8 * a + 8, :],
                                    wv[:, 8 * a:8 * a + 8, :])
            for h in range(HPC):
                nc.scalar.dma_start(wp_s[h][:], wp[h])
            for a in range(8):
                nc.sync.dma_start(xbs[1][:, 2 * a:2 * a + 2, :],
                                  xT[1, :, 2 * a:2 * a + 2, :])
            # valid (1.0) where q >= k for the transposed [k, q] diag block
            make_upper_triangular(nc, trimask[:], val=1.0, diag=True)
            make_identity(nc, ident[:])

            qkTs = [None] * B
            v_alls = [None] * B

            def alloc_batch(b):
                qkTs[b] = qkpool.tile([128, 4, T], bf16, tag="qkT",
                                      name="qkT")
                v_alls[b] = vpool.tile([128, TB, HPC, D + 1], bf16, tag="v",
                                       name="v_all")
                nc.vector.memset(v_alls[b][:, :, :, D:D + 1], 1.0)

            with nc.named_scope("prologue"):
                alloc_batch(0)
                for idx in range(2 * 4):
                    emit_qk_group(xbs[0], qkTs[0], idx, order=1)
                for tb in range(TB):
                    emit_v_group(xbs[0], v_alls[0], tb)

            # ---- pipelined batches ----------------------------------------
            for b in range(B):
                t0 = b * T
                attTs = [attpool.tile([128, TB, T], bf16, tag="attT",
                                      name=f"attT{h}") for h in range(HPC)]
                yTs = [ytpool.tile([128, T], bf16, tag="yT",
                                   name=f"yT{h}") for h in range(HPC)]
                if b + 1 < B:
                    alloc_batch(b + 1)
                if b + 2 < B:
                    xbs[b + 2] = xpool.tile([128, CCH, T], bf16, tag="xb",
                                            name="xb")

                with nc.named_scope(f"batch{b}"):
                    for s in range(TB):
                        emit_score_row(qkTs[b], 0, attTs[0], s)
                        if b + 1 < B:
                            emit_qk_group(xbs[b + 1], qkTs[b + 1], s)
                        emit_score_row(qkTs[b], 1, attTs[1], s)
                        if b == 3 and 0 < s < TB - 1:
                            # no next-batch QK/V filler: use the projection
                            # as cover for the exp -> mask -> att@v chain
                            emit_proj(t0, yTs, s - 1)
                        ch0 = emit_av(attTs[0], v_alls[b], 0, s)
                        if b + 1 < B:
                            emit_v_group(xbs[b + 1], v_alls[b + 1], s)
                        ch1 = emit_av(attTs[1], v_alls[b], 1, s)
                        emit_transpose(*ch0, yTs[0], s)
                        emit_transpose(*ch1, yTs[1], s)
                        if b < 3 and s > 0:
                            emit_proj(t0, yTs, s - 1)
                        elif b == 3 and s == TB - 1:
                            # last stage: att@v chain first so the kernel
                            # tail (y(7) -> proj -> store) starts ASAP
                            emit_proj(t0, yTs, s - 1)
                        if b + 2 < B:
                            nc.sync.dma_start(
                                xbs[b + 2][:, 2 * s:2 * s + 2, :],
                                xT[b + 2, :, 2 * s:2 * s + 2, :])
                    emit_proj(t0, yTs, TB - 1, split_dma=(b == 3))

    nc.compile()
    return nc


def kernel(x, w_attn_q, s_w_attn, z_w_attn, b_attn_q, s_b_attn, z_b_attn,
           w_proj_q, s_w_proj, z_w_proj, b_proj_q, s_b_proj, z_b_proj):
    from concourse.bass_utils import run_bass_kernel_spmd

    x = np.asarray(x, np.float32)
    w_attn_q = np.asarray(w_attn_q)
    b_attn_q = np.asarray(b_attn_q)
    w_proj_q = np.asarray(w_proj_q)
    b_proj_q = np.asarray(b_proj_q)
    s_wa = float(s_w_attn)
    s_ba = float(s_b_attn)
    s_wp = float(s_w_proj)
    s_bp = float(s_b_proj)

    # integer-valued dequantized weights; |value| <= 255 so exact in bf16
    wa_int = (w_attn_q.astype(np.int32) - int(z_w_attn)).astype(np.float32)
    wp_int = (w_proj_q.astype(np.int32) - int(z_w_proj)).astype(np.float32)
    ba_true = s_ba * (b_attn_q.astype(np.int32) - int(z_b_attn)).astype(np.float32)
    bp_true = s_bp * (b_proj_q.astype(np.int32) - int(z_b_proj)).astype(np.float32)

    xT_np = np.ascontiguousarray(
        np.swapaxes(x, 1, 2).reshape(B, CCH, 128, T).transpose(0, 2, 1, 3)
    ).astype(BF16)                                   # [B, 128, CCH, T]

    key = (s_wa,)
    if key not in _CACHE:
        _CACHE[key] = _build_program(s_wa)
    nc = _CACHE[key]

    in_maps = []
    for c in range(NCORES):
        r0 = c * CS                    # q rows for this core's heads
        wq = wa_int[r0:r0 + CS]                    # [256, C]
        wk = wa_int[C + r0:C + r0 + CS]
        wv_rows = wa_int[2 * C + r0:2 * C + r0 + CS]
        wqk_np = np.ascontiguousarray(
            np.concatenate([wq, wk], axis=0).T       # [C, 512]
            .reshape(CCH, 128, 4 * 128).transpose(1, 0, 2)
        ).astype(BF16)                               # [128, CCH, 512]
        wv_np = np.ascontiguousarray(
            wv_rows.T.reshape(CCH, 128, CS).transpose(1, 0, 2)
        ).astype(BF16)                               # [128, CCH, 256]
        wp_np = np.ascontiguousarray(
            wp_int[:, r0:r0 + CS].T                  # [256, C]
        ).reshape(HPC, 128, C).astype(BF16)
        bq = ba_true[r0:r0 + CS]
        bk = ba_true[C + r0:C + r0 + CS]
        bqk_np = np.ascontiguousarray(
            np.concatenate([bq, bk]).reshape(4, 128).T  # [128, 4]
        ).astype(np.float32)
        in_maps.append({
            "xT": xT_np,
            "wqk": wqk_np,
            "wv": wv_np,
            "wp": wp_np,
            "bqk": bqk_np,
        })

    res = run_bass_kernel_spmd(nc, in_maps, core_ids=list(range(NCORES)))

    acc = np.zeros((BT, C), np.float64)
    for c in range(NCORES):
        acc += res.results[c]["partial"].astype(np.float64)
    # v and w_proj were used unscaled on device; apply the commuting scales
    # here. The v-bias passes through normalized attention as a constant row;
    # add it (and the c_proj bias) here, exactly, in fp64->fp32.
    bv_true = ba_true[2 * C:3 * C]
    bv_fold = (s_wp * (bv_true.astype(np.float64) @ wp_int.astype(np.float64).T))
    out = (s_wa * s_wp) * acc + bv_fold[None, :] + bp_true.astype(np.float64)[None, :]
    return out.reshape(B, T, C).astype(np.float32)


# revision 21
# speedup vs baseline: 1.0045x; 1.0045x over previous
"""Trainium2 Bass kernel for quantized causal self-attention.

Sharding: Megatron-style tensor parallelism over heads. 16 heads are split
across 8 NeuronCores (2 heads/core). Each core computes, for all 4 batches:
  - its QKV head-slice projection (x @ w_slice^T, int8 weights held exactly
    in bf16, quant scales folded into epilogues / host),
  - causal attention for its 2 heads (transposed-scores layout, exp without
    max-subtraction - scores are bounded ~5 for this model family),
  - a partial output projection against its column slice of w_proj.
The host sums the 8 partial projections, applies the commuting quant scales
(s_w_attn * s_w_proj), and adds the biases that commute out of the linear
ops (c_proj bias, and the v-bias term which passes through softmax-normalized
attention as a constant row).

Schedule: fully software-pipelined across batches. Every stage s of batch b
interleaves, in PE program order, work with no mutual dependencies:
  scores(b,h0,row s) | QK(b+1, group s) | scores(b,h1,row s) |
  att@v(b,h0,qb=s) | V(b+1, group s) | att@v(b,h1,qb=s) |
  y-transposes | c_proj(b, tb=s-1)
so the tensor engine never waits on the scalar-engine exp or the DVE drain
chains. The causal-mask multiply runs on the otherwise-idle Pool engine and
the c_proj drains alternate DVE/ACT. Partials are written in bf16.
"""

import numpy as np
import ml_dtypes

B, T, C, H, D = 4, 1024, 2048, 16, 128
NCORES = 8
HPC = H // NCORES          # heads per core = 2
CS = HPC * D               # per-core head feature slice = 256
BT = B * T                 # 4096 tokens
TB = T // 128              # 8 token blocks per batch
CCH = C // 128             # 16 contraction chunks

BF16 = ml_dtypes.bfloat16

_CACHE = {}


def _build_program(s_wa: float):
    import concourse.tile as tile
    from concourse import bacc, mybir
    from concourse.masks import make_identity, make_upper_triangular

    f32 = mybir.dt.float32
    bf16 = mybir.dt.bfloat16
    AF = mybir.ActivationFunctionType
    inv_sqrt_d = 1.0 / float(np.sqrt(D))

    nc = bacc.Bacc("TRN2", target_bir_lowering=False, debug=False)

    # all partition-major so SBUF-shaped multi-chunk DMAs need no transpose
    xT = nc.dram_tensor("xT", [B, 128, CCH, T], bf16, kind="ExternalInput")
    wqk = nc.dram_tensor("wqk", [128, CCH, 4 * 128], bf16, kind="ExternalInput")
    wv = nc.dram_tensor("wv", [128, CCH, CS], bf16, kind="ExternalInput")
    wp = nc.dram_tensor("wp", [HPC, 128, C], bf16, kind="ExternalInput")
    bqk = nc.dram_tensor("bqk", [128, 4], f32, kind="ExternalInput")
    partial = nc.dram_tensor("partial", [BT, C], bf16, kind="ExternalOutput")

    with tile.TileContext(nc) as tc:
        with (
            tc.tile_pool(name="singles", bufs=1) as singles,
            tc.tile_pool(name="xpool", bufs=2) as xpool,
            tc.tile_pool(name="qkpool", bufs=3) as qkpool,
            tc.tile_pool(name="vpool", bufs=3) as vpool,
            tc.tile_pool(name="attpool", bufs=3) as attpool,
            tc.tile_pool(name="ytpool", bufs=4) as ytpool,
            tc.tile_pool(name="ypool", bufs=3) as ypool,
            tc.tile_pool(name="rlpool", bufs=4) as rlpool,
            tc.tile_pool(name="outpool", bufs=4) as outpool,
            # 3 banks shared by QK / V / c_proj matmul groups: their drains
            # complete quickly, so a 3-deep ring never stalls the PE
            tc.tile_pool(name="psbig", bufs=3, space="PSUM") as psbig,
            # score rows: up to 4 chunks in flight per stage while the
            # scalar engine works through the exps
            tc.tile_pool(name="psatt", bufs=3, space="PSUM") as psatt,
            # att@v accumulators (129 cols + bf16 transpose target)
            tc.tile_pool(name="pssm", bufs=2, space="PSUM") as pssm,
        ):
            # weights / constants, resident for the whole kernel
            wqk_s = singles.tile([128, CCH, 4 * 128], bf16, tag="wqk")
            wv_s = singles.tile([128, CCH, CS], bf16, tag="wv")
            wp_s = [singles.tile([128, C], bf16, tag=f"wp{h}", name=f"wp_s{h}")
                    for h in range(HPC)]
            bqk_s = singles.tile([128, 4], f32, tag="bqk")
            trimask = singles.tile([128, 128], bf16, tag="trimask")
            ident = singles.tile([128, 128], bf16, tag="ident")

            # ---- emitters --------------------------------------------------
            def emit_qk_group(xb, qkT, idx, order=2):
                # idx -> (ob, th); ob: 0 = q head0, 1 = q head1, 2 = k
                # head0, 3 = k head1; th: halves of the 1024 tokens.
                # order=1 walks th-major (prologue: matches DMA arrival)
                if order == 2:
                    ob, th = idx // 2, idx % 2
                else:
                    ob, th = idx % 4, idx // 4
                ps = psbig.tile([128, 512], f32, tag="ps", name="ps")
                for g in range(CCH):
                    nc.tensor.matmul(
                        ps[:],
                        wqk_s[:, g, ob * 128:(ob + 1) * 128],
                        xb[:, g, th * 512:(th + 1) * 512],
                        start=(g == 0),
                        stop=(g == CCH - 1),
                    )
                nc.vector.tensor_scalar(
                    qkT[:, ob, th * 512:(th + 1) * 512],
                    ps[:],
                    s_wa,
                    bqk_s[:, ob:ob + 1],
                    mybir.AluOpType.mult,
                    mybir.AluOpType.add,
                )

            def emit_v_group(xb, v_all, tb):
                ps = psbig.tile([128, CS], f32, tag="ps", name="ps")
                for g in range(CCH):
                    nc.tensor.matmul(
                        ps[:],
                        xb[:, g, tb * 128:(tb + 1) * 128],
                        wv_s[:, g, :],
                        start=(g == 0),
                        stop=(g == CCH - 1),
                    )
                nc.vector.tensor_copy(
                    v_all[:, tb, :, 0:D],
                    ps[:].rearrange("p (h d) -> p h d", h=HPC),
                )

            def emit_score_row(qkT, h, attT, kb):
                width = T - kb * 128
                off = 0
                while off < width:
                    w = min(512, width - off)
                    ps = psatt.tile([128, 512], f32, tag="ps", name="ps")
                    nc.tensor.matmul(
                        ps[:, 0:w],
                        qkT[:, 2 + h, kb * 128:(kb + 1) * 128],
                        qkT[:, h, kb * 128 + off:kb * 128 + off + w],
                    )
                    nc.scalar.activation(
                        attT[:, kb, off:off + w],
                        ps[:, 0:w],
                        AF.Exp,
                        scale=inv_sqrt_d,
                    )
                    off += w
                # causal mask on the diagonal block (multiplicative), on the
                # otherwise-idle Pool engine
                nc.gpsimd.tensor_mul(
                    attT[:, kb, 0:128], attT[:, kb, 0:128], trimask[:],
                )

            def emit_av(attT, v_all, h, qb):
                # cols 0:D = y accum, col D = row-sum (ones column of v);
                # the bf16 view of f32 cols 192:256 later receives the PE
                # transpose so no extra PSUM slot is consumed for it
                psy = pssm.tile([128, 256], f32, tag="psy", name="psy")
                for kb in range(qb + 1):
                    nc.tensor.matmul(
                        psy[:, 0:D + 1],
                        attT[:, kb, (qb - kb) * 128:(qb - kb) * 128 + 128],
                        v_all[:, kb, h, :],
                        start=(kb == 0),
                        stop=(kb == qb),
                    )
                rl = rlpool.tile([128, 1], f32, tag="rl", name="rl")
                nc.vector.reciprocal(rl[:], psy[:, D:D + 1])
                ysb = ypool.tile([128, 128], bf16, tag="ysb", name="ysb")
                nc.vector.tensor_scalar_mul(ysb[:], psy[:, 0:D], rl[:])
                return psy, ysb

            def emit_transpose(psy, ysb, yT, qb):
                pst = psy[:, 192:256].bitcast(bf16)
                nc.tensor.transpose(pst, ysb[:], ident[:])
                nc.vector.tensor_copy(yT[:, qb * 128:(qb + 1) * 128], pst)

            def emit_proj(t0, yTs, tb, obs=(0, 1, 2, 3)):
                # ob 0/1 drain+dispatch on DVE+SP, ob 2/3 on ACT, so the
                # engine queues stay balanced
                for ob in obs:
                    ps = psbig.tile([128, 512], f32, tag="ps", name="ps")
                    for h in range(HPC):
                        nc.tensor.matmul(
                            ps[:],
                            yTs[h][:, tb * 128:(tb + 1) * 128],
                            wp_s[h][:, ob * 512:(ob + 1) * 512],
                            start=(h == 0),
                            stop=(h == HPC - 1),
                        )
                    po = outpool.tile([128, 512], bf16, tag="po", name="po")
                    dst = partial[t0 + tb * 128:t0 + (tb + 1) * 128,
                                  ob * 512:(ob + 1) * 512]
                    if ob < 2:
                        nc.vector.tensor_copy(po[:], ps[:])
                    else:
                        nc.scalar.copy(po[:], ps[:])
                    nc.sync.dma_start(dst, po[:])

            # ---- prologue --------------------------------------------------
            # weights dispatch from the (idle) ACT hardware-DGE queue, x from
            # SP. Slice the loads so the first QK group (ob=0, th=0) only
            # needs its own wqk column block + the th=0 half of x: ~2.6MB of
            # HBM traffic instead of 6.2MB before the PE gets rolling.
            xbs = [None] * B
            xbs[0] = xpool.tile([128, CCH, T], bf16, tag="xb", name="xb")
            xbs[1] = xpool.tile([128, CCH, T], bf16, tag="xb", name="xb")
            for ob in range(4):
                nc.scalar.dma_start(wqk_s[:, :, ob * 128:(ob + 1) * 128],
                                    wqk[:, :, ob * 128:(ob + 1) * 128])
            nc.scalar.dma_start(bqk_s[:], bqk[:])
            for th in range(2):
                for a in range(4):
                    nc.sync.dma_start(
                        xbs[0][:, 4 * a:4 * a + 4, th * 512:(th + 1) * 512],
                        xT[0, :, 4 * a:4 * a + 4, th * 512:(th + 1) * 512])
            for a in range(2):
                nc.scalar.dma_start(wv_s[:, 8 * a:8 * a + 8, :],
                                    wv[:, 8 * a:8 * a + 8, :])
            for h in range(HPC):
                nc.scalar.dma_start(wp_s[h][:], wp[h])
            for a in range(8):
                nc.sync.dma_start(xbs[1][:, 2 * a:2 * a + 2, :],
                                  xT[1, :, 2 * a:2 * a + 2, :])
            # valid (1.0) where q >= k for the transposed [k, q] diag block
            make_upper_triangular(nc, trimask[:], val=1.0, diag=True)
            make_identity(nc, ident[:])

            qkTs = [None] * B
            v_alls = [None] * B

            def alloc_batch(b):
                qkTs[b] = qkpool.tile([128, 4, T], bf16, tag="qkT",
                                      name="qkT")
                v_alls[b] = vpool.tile([128, TB, HPC, D + 1], bf16, tag="v",
                                       name="v_all")
                nc.vector.memset(v_alls[b][:, :, :, D:D + 1], 1.0)

            with nc.named_scope("prologue"):
                alloc_batch(0)
                for idx in range(2 * 4):
                    emit_qk_group(xbs[0], qkTs[0], idx, order=1)
                for tb in range(TB):
                    emit_v_group(xbs[0], v_alls[0], tb)

            # ---- pipelined batches ----------------------------------------
            for b in range(B):
                t0 = b * T
                attTs = [attpool.tile([128, TB, T], bf16, tag="attT",
                                      name=f"attT{h}") for h in range(HPC)]
                yTs = [ytpool.tile([128, T], bf16, tag="yT",
                                   name=f"yT{h}") for h in range(HPC)]
                if b + 1 < B:
                    alloc_batch(b + 1)
                if b + 2 < B:
                    xbs[b + 2] = xpool.tile([128, CCH, T], bf16, tag="xb",
                                            name="xb")

                with nc.named_scope(f"batch{b}"):
                    for s in range(TB):
                        emit_score_row(qkTs[b], 0, attTs[0], s)
                        if b + 1 < B:
                            emit_qk_group(xbs[b + 1], qkTs[b + 1], s)
                        emit_score_row(qkTs[b], 1, attTs[1], s)
                        if b == 3 and s > 0:
                            # no next-batch QK/V filler: use the projection
                            # as cover for the exp -> mask -> att@v chain
                            emit_proj(t0, yTs, s - 1)
                        ch0 = emit_av(attTs[0], v_alls[b], 0, s)
                        if b + 1 < B:
                            emit_v_group(xbs[b + 1], v_alls[b + 1], s)
                        ch1 = emit_av(attTs[1], v_alls[b], 1, s)
                        emit_transpose(*ch0, yTs[0], s)
                        emit_transpose(*ch1, yTs[1], s)
                        if b < 3 and s > 0:
                            emit_proj(t0, yTs, s - 1)
                        if b + 2 < B:
                            nc.sync.dma_start(
                                xbs[b + 2][:, 2 * s:2 * s + 2, :],
                                xT[b + 2, :, 2 * s:2 * s + 2, :])
                    emit_proj(t0, yTs, TB - 1)

    nc.compile()
    return nc


def kernel(x, w_attn_q, s_w_attn, z_w_attn, b_attn_q, s_b_attn, z_b_attn,
           w_proj_q, s_w_proj, z_w_proj, b_proj_q, s_b_proj, z_b_proj):
    from concourse.bass_utils import run_bass_kernel_spmd

    x = np.asarray(x, np.float32)
    w_attn_q = np.asarray(w_attn_q)
    b_attn_q = np.asarray(b_attn_q)
    w_proj_q = np.asarray(w_proj_q)
    b_proj_q = np.asarray(b_proj_q)
    s_wa = float(s_w_attn)
    s_ba = float(s_b_attn)
    s_wp = float(s_w_proj)
    s_bp = float(s_b_proj)

    # integer-valued dequantized weights; |value| <= 255 so exact in bf16
    wa_int = (w_attn_q.astype(np.int32) - int(z_w_attn)).astype(np.float32)
    wp_int = (w_proj_q.astype(np.int32) - int(z_w_proj)).astype(np.float32)
    ba_true = s_ba * (b_attn_q.astype(np.int32) - int(z_b_attn)).astype(np.float32)
    bp_true = s_bp * (b_proj_q.astype(np.int32) - int(z_b_proj)).astype(np.float32)

    xT_np = np.ascontiguousarray(
        np.swapaxes(x, 1, 2).reshape(B, CCH, 128, T).transpose(0, 2, 1, 3)
    ).astype(BF16)                                   # [B, 128, CCH, T]

    key = (s_wa,)
    if key not in _CACHE:
        _CACHE[key] = _build_program(s_wa)
    nc = _CACHE[key]

    in_maps = []
    for c in range(NCORES):
        r0 = c * CS                    # q rows for this core's heads
        wq = wa_int[r0:r0 + CS]                    # [256, C]
        wk = wa_int[C + r0:C + r0 + CS]
        wv_rows = wa_int[2 * C + r0:2 * C + r0 + CS]
        wqk_np = np.ascontiguousarray(
            np.concatenate([wq, wk], axis=0).T       # [C, 512]
            .reshape(CCH, 128, 4 * 128).transpose(1, 0, 2)
        ).astype(BF16)                               # [128, CCH, 512]
        wv_np = np.ascontiguousarray(
            wv_rows.T.reshape(CCH, 128, CS).transpose(1, 0, 2)
        ).astype(BF16)                               # [128, CCH, 256]
        wp_np = np.ascontiguousarray(
            wp_int[:, r0:r0 + CS].T                  # [256, C]
        ).reshape(HPC, 128, C).astype(BF16)
        bq = ba_true[r0:r0 + CS]
        bk = ba_true[C + r0:C + r0 + CS]
        bqk_np = np.ascontiguousarray(
            np.concatenate([bq, bk]).reshape(4, 128).T  # [128, 4]
        ).astype(np.float32)
        in_maps.append({
            "xT": xT_np,
            "wqk": wqk_np,
            "wv": wv_np,
            "wp": wp_np,
            "bqk": bqk_np,
        })

    res = run_bass_kernel_spmd(nc, in_maps, core_ids=list(range(NCORES)))

    acc = np.zeros((BT, C), np.float64)
    for c in range(NCORES):
        acc += res.results[c]["partial"].astype(np.float64)
    # v and w_proj were used unscaled on device; apply the commuting scales
    # here. The v-bias passes through normalized attention as a constant row;
    # add it (and the c_proj bias) here, exactly, in fp64->fp32.
    bv_true = ba_true[2 * C:3 * C]
    bv_fold = (s_wp * (bv_true.astype(np.float64) @ wp_int.astype(np.float64).T))
    out = (s_wa * s_wp) * acc + bv_fold[None, :] + bp_true.astype(np.float64)[None, :]
    return out.reshape(B, T, C).astype(np.float32)
